# revision 16
# baseline (speedup 1.0000x reference)
"""Trainium2 Bass kernel for nn_Custom_Self_Attention_28621662060731.

Reference semantics (all "3x3 convs" act on width-1 reshaped tensors, so they
are 3-tap 1D convs along the flattened position axis):

    k_i  = conv3(class_input)        [64, 4096]
    sk_i = conv3(saved_class_input)  [64, 4096]
    q_i  = conv3(input)              [64, 16384]
    memory_map = sigmoid(c1 * cos(sk1,k1) + c2 * cos(sk2,k2) + cb1)  [4096, 4096]
    r_i  = onehot @ cos(k_i, q_i) / counts                            [4, 16384]
    result_map = sigmoid(cc2 @ [r1;r2] + cb2)                         [1,4,128,128]

Key algebraic collapse: the [4096, 16384] attention maps are never
materialized.  r_i = (onehot/counts @ k_i_norm^T) @ q_i_norm, a [4,64]
matrix applied to the normalized queries.

Sharding: memory_map rows (S2) and query pixels (HW) both sharded 8-way;
keys/weights replicated.  Each core computes a [512, 4096] memory_map shard
and a [4, 2048] result shard.  No collectives.
"""

import numpy as np

import concourse.bass as bass
import concourse.bacc as bacc
import concourse.mybir as mybir
from concourse.tile import TileContext
from concourse.bass_utils import run_bass_kernel_spmd

NCORES = 8
IN_DIM, ATTN, NCLS = 128, 64, 4
H = W = 128
HW = H * W                 # 16384
S1 = S2 = 4096
QSH = HW // NCORES         # 2048 query pixels per core
MSH = S2 // NCORES         # 512 memory-map rows per core
CH = 512                   # column chunk (one PSUM bank of fp32)
F32 = mybir.dt.float32
AF = mybir.ActivationFunctionType
MULT = mybir.AluOpType.mult


def build_program(stage: int = 99) -> bass.Bass:
    """stage limits how much of the pipeline is emitted (for HW bisection):
    0: input DMAs + copy-out   1: +key chunks   2: +A/B   3: +memory map
    99: full kernel."""
    nc = bacc.Bacc()

    # chunked inputs: chunk n occupies cols [514n, 514n+514) = 512 cols + halo
    cls = nc.declare_dram_parameter("cls", [128, 8 * (CH + 2)], F32, isOutput=False)
    scls = nc.declare_dram_parameter("scls", [128, CH + 2], F32, isOutput=False)
    inp = nc.declare_dram_parameter("inp", [128, 4 * (CH + 2)], F32, isOutput=False)
    # packed constants, see _prep_in_maps for layout
    wcst = nc.declare_dram_parameter("wcst", [128, 768], F32, isOutput=False)
    rcst = nc.declare_dram_parameter("rcst", [1, 640], F32, isOutput=False)
    ccst = nc.declare_dram_parameter("ccst", [128, 2], F32, isOutput=False)
    c4 = nc.declare_dram_parameter("c4", [4, 12], F32, isOutput=False)

    mm_out = nc.declare_dram_parameter("mm_out", [MSH, S1], F32, isOutput=True)
    res_out = nc.declare_dram_parameter("res_out", [4, QSH], F32, isOutput=True)

    with TileContext(nc) as tc:
        with (
            tc.tile_pool(name="const", bufs=1) as constp,
            tc.tile_pool(name="io", bufs=1) as iop,
            tc.tile_pool(name="work", bufs=3) as wp,
            tc.tile_pool(name="mmo", bufs=2) as mmop,
            tc.tile_pool(name="pacc", bufs=1, space="PSUM") as paccp,
            tc.tile_pool(name="praw", bufs=2, space="PSUM") as prawp,
            tc.tile_pool(name="pshare", bufs=2, space="PSUM") as pshp,
            tc.tile_pool(name="psmall", bufs=2, space="PSUM") as psmp,
        ):
            wcst_s = constp.tile([128, 768], F32, name="wcst_s")
            nc.sync.dma_start(out=wcst_s, in_=wcst[:, :])
            rcst_s = constp.tile([1, 640], F32, name="rcst_s")
            nc.sync.dma_start(out=rcst_s, in_=rcst[:, :])
            ccst_s = constp.tile([128, 2], F32, name="ccst_s")
            nc.sync.dma_start(out=ccst_s, in_=ccst[:, :])
            c4_s = constp.tile([4, 12], F32, name="c4_s")
            nc.sync.dma_start(out=c4_s, in_=c4[:, :])

            scls_s = iop.tile([128, CH + 2], F32, name="scls_s")
            nc.sync.dma_start(out=scls_s, in_=scls[:, :])
            cls_t = []
            for n in range(8):
                t = iop.tile([128, CH + 2], F32, name=f"cls_t{n}")
                nc.sync.dma_start(out=t, in_=cls[:, (CH + 2) * n:(CH + 2) * (n + 1)])
                cls_t.append(t)
            inp_t = []
            for m in range(4):
                t = iop.tile([128, CH + 2], F32, name=f"inp_t{m}")
                nc.sync.dma_start(out=t, in_=inp[:, (CH + 2) * m:(CH + 2) * (m + 1)])
                inp_t.append(t)

            # constant views
            def w1t(t):
                return wcst_s[:, 64 * t:64 * t + 64]

            def w2t(t):
                return wcst_s[:, 192 + 64 * t:192 + 64 * t + 64]

            mct = wcst_s[:, 384:512]        # [128, 128] Msc^T chunks
            bd = wcst_s[:, 512:640]         # [128, 128] blockdiag ones
            # [64, 64] identity replicated in both partition halves
            ident = [wcst_s[0:64, 640:704], wcst_s[64:128, 640:704]]
            bstack = rcst_s[0:1, 0:128]     # b1 ++ b2
            ones512 = rcst_s[0:1, 128:640]
            cvec = ccst_s[:, 0:1]           # c1 rows 0:64, c2 rows 64:128
            cb1v = ccst_s[:, 1:2]           # cc1_b broadcast
            cc2aT = c4_s[0:4, 0:4]
            cc2bT = c4_s[0:4, 4:8]
            cb2v = c4_s[0:4, 8:9]

            # stacked normalized keys/queries: rows 0:64 conv1, 64:128 conv2
            kc = iop.tile([128, S1], F32, name="kc")
            skc = iop.tile([128, CH], F32, name="skc")
            qc = iop.tile([128, QSH], F32, name="qc")

            def key_chunk(src, dst, col0, scale_cvec):
                """conv both attn heads on 512 positions, column-normalize,
                write stacked normalized result into dst[:, col0:col0+CH]."""
                pr = prawp.tile([128, CH], F32, name="pr", tag="pr")
                for t in range(3):
                    nc.tensor.matmul(pr[0:64, :], lhsT=w1t(t),
                                     rhs=src[:, t:t + CH],
                                     start=(t == 0), stop=False,
                                     skip_group_check=True)
                for t in range(3):
                    nc.tensor.matmul(pr[64:128, :], lhsT=w2t(t),
                                     rhs=src[:, t:t + CH],
                                     start=(t == 0), stop=False,
                                     skip_group_check=True)
                # conv bias as rank-1 accumulate: b ⊗ ones
                nc.tensor.matmul(pr[:, :], lhsT=bstack, rhs=ones512,
                                 start=False, stop=True, skip_group_check=True)
                sq = wp.tile([128, CH], F32, name="sq", tag="sq")
                nc.scalar.square(sq, pr)
                # per-column sum of squares broadcast to all partitions
                pn = pshp.tile([128, CH], F32, name="pn", tag="big")
                nc.tensor.matmul(pn, lhsT=bd, rhs=sq, start=True, stop=True)
                rec = wp.tile([128, CH], F32, name="rec", tag="rec")
                nc.vector.reciprocal(rec, pn)
                inv = wp.tile([128, CH], F32, name="inv", tag="inv")
                nc.scalar.sqrt(inv, rec)   # 1/sqrt(ss) = sqrt(1/ss)
                if scale_cvec:
                    nc.vector.scalar_tensor_tensor(
                        out=dst[:, col0:col0 + CH], in0=pr, scalar=cvec,
                        in1=inv, op0=MULT, op1=MULT)
                else:
                    nc.vector.tensor_mul(dst[:, col0:col0 + CH], pr, inv)

            if stage == 0:
                nc.sync.dma_start(out=mm_out[0:128, 0:CH + 2], in_=cls_t[0])

            if stage >= 1:
                # saved keys (this core's 512 rows), scaled by cc1_w
                key_chunk(scls_s, skc, 0, True)
                # full keys, replicated
                for n in range(8):
                    key_chunk(cls_t[n], kc, CH * n, False)
            if stage == 1:
                nc.sync.dma_start(out=mm_out[0:128, :], in_=kc)
                nc.sync.dma_start(out=mm_out[128:256, 0:CH], in_=skc)

            if stage >= 2:
                # A matrices: A_j = (onehot/counts) @ k_j_norm^T, [4, 64|64]
                # 1) transpose all normalized keys into knT_all [s, attn|attn]
                knT_all = iop.tile([128, 32 * 128], F32, name="knT_all")
                for i in range(32):
                    for j in range(2):
                        ptr = psmp.tile([128, 64], F32, name="ptr", tag="small")
                        nc.tensor.transpose(
                            ptr, kc[64 * j:64 * j + 64, 128 * i:128 * i + 128],
                            ident[j])
                        nc.vector.tensor_copy(
                            knT_all[:, 128 * i + 64 * j:128 * i + 64 * j + 64],
                            ptr)
                # 2) uninterrupted accumulation over the 32 s-chunks
                A_ps = paccp.tile([4, 128], F32, name="A_ps")
                for i in range(32):
                    nc.tensor.matmul(A_ps, lhsT=mct[:, 4 * i:4 * i + 4],
                                     rhs=knT_all[:, 128 * i:128 * i + 128],
                                     start=(i == 0), stop=(i == 31),
                                     skip_group_check=True)

                # B_j^T = A_j^T @ cc2_j^T, stacked [128, 4]
                A_sb = iop.tile([4, 128], F32, name="A_sb")
                nc.scalar.copy(A_sb, A_ps)
                B_ps = psmp.tile([128, 4], F32, name="B_ps", tag="small")
                nc.tensor.matmul(B_ps[0:64, :], lhsT=A_sb[0:4, 0:64],
                                 rhs=cc2aT, start=True, stop=True,
                                 skip_group_check=True)
                nc.tensor.matmul(B_ps[64:128, :], lhsT=A_sb[0:4, 64:128],
                                 rhs=cc2bT, start=True, stop=True,
                                 skip_group_check=True)
                Bc_sb = iop.tile([128, 4], F32, name="Bc_sb")
                nc.vector.tensor_copy(Bc_sb, B_ps)
            if stage == 2:
                nc.sync.dma_start(out=res_out[0:4, 0:128], in_=A_sb)

            if stage >= 3:
                # memory map shard: sigmoid(skc^T @ kc + cb1)  [512, 4096]
                for m in range(4):
                    mm_sb = mmop.tile([128, S1], F32, name="mm_sb", tag="mm_sb")
                    for n in range(8):
                        pb = pshp.tile([128, CH], F32, name="pb", tag="big")
                        nc.tensor.matmul(pb,
                                         lhsT=skc[:, 128 * m:128 * m + 128],
                                         rhs=kc[:, CH * n:CH * n + CH],
                                         start=True, stop=True)
                        nc.scalar.activation(mm_sb[:, CH * n:CH * n + CH], pb,
                                             AF.Sigmoid, bias=cb1v)
                    nc.sync.dma_start(out=mm_out[128 * m:128 * m + 128, :],
                                      in_=mm_sb)

            if stage >= 4:
                # queries for this core's 2048 pixels
                for m in range(4):
                    key_chunk(inp_t[m], qc, CH * m, False)

                # result shard: sigmoid(Bc^T @ qc + cb2)  [4, 2048]
                res_sb = iop.tile([4, QSH], F32, name="res_sb")
                for m in range(4):
                    pres = psmp.tile([4, CH], F32, name="pres", tag="small")
                    nc.tensor.matmul(pres, lhsT=Bc_sb[:, 0:4],
                                     rhs=qc[:, CH * m:CH * m + CH],
                                     start=True, stop=True)
                    nc.scalar.activation(res_sb[:, CH * m:CH * m + CH], pres,
                                         AF.Sigmoid, bias=cb2v)
                nc.sync.dma_start(out=res_out[:, :], in_=res_sb)

    nc.finalize()
    return nc


def _chunk_with_halo(x_pad: np.ndarray, nchunks: int) -> np.ndarray:
    """x_pad [128, L+2] zero-padded; -> [128, nchunks*(CH+2)] where chunk n
    is x_pad[:, CH*n : CH*n+CH+2]."""
    out = np.empty((128, nchunks * (CH + 2)), np.float32)
    for n in range(nchunks):
        out[:, (CH + 2) * n:(CH + 2) * (n + 1)] = x_pad[:, CH * n:CH * n + CH + 2]
    return out


def _prep_in_maps(inputs: dict) -> list[dict]:
    inp_full = np.ascontiguousarray(
        inputs["input"].reshape(IN_DIM, HW), dtype=np.float32)
    cls_full = np.ascontiguousarray(
        inputs["class_input"].reshape(IN_DIM, S1), dtype=np.float32)
    scls_full = np.ascontiguousarray(
        inputs["saved_class_input"].reshape(IN_DIM, S2), dtype=np.float32)
    w1 = np.asarray(inputs["w1"], np.float32)
    w2 = np.asarray(inputs["w2"], np.float32)
    b1 = np.asarray(inputs["b1"], np.float32)
    b2 = np.asarray(inputs["b2"], np.float32)
    cc1_w = np.asarray(inputs["cc1_w"], np.float32)
    cc1_b = np.asarray(inputs["cc1_b"], np.float32)
    cc2_w = np.asarray(inputs["cc2_w"], np.float32)
    cc2_b = np.asarray(inputs["cc2_b"], np.float32)
    lbl = np.asarray(inputs["class_label"])

    cls_pad = np.zeros((IN_DIM, S1 + 2), np.float32)
    cls_pad[:, 1:S1 + 1] = cls_full
    cls_d = _chunk_with_halo(cls_pad, 8)

    onehot = (lbl[None, :] == np.arange(NCLS)[:, None]).astype(np.float32)
    msc = onehot / onehot.sum(1, keepdims=True)          # [4, 4096]

    wcst = np.zeros((128, 768), np.float32)
    for t in range(3):
        wcst[:, 64 * t:64 * t + 64] = w1[:, :, t, 1].T
        wcst[:, 192 + 64 * t:192 + 64 * t + 64] = w2[:, :, t, 1].T
    # mct[p, 4i+c] = msc[c, 128i+p]
    wcst[:, 384:512] = msc.reshape(NCLS, 32, 128).transpose(2, 1, 0).reshape(128, 128)
    wcst[0:64, 512:576] = 1.0
    wcst[64:128, 576:640] = 1.0
    wcst[0:64, 640:704] = np.eye(64, dtype=np.float32)
    wcst[64:128, 640:704] = np.eye(64, dtype=np.float32)

    rcst = np.zeros((1, 640), np.float32)
    rcst[0, 0:64] = b1
    rcst[0, 64:128] = b2
    rcst[0, 128:640] = 1.0

    ccst = np.zeros((128, 2), np.float32)
    ccst[0:64, 0] = cc1_w[0, 0, 0, 0]
    ccst[64:128, 0] = cc1_w[0, 1, 0, 0]
    ccst[:, 1] = cc1_b[0]

    c4 = np.zeros((4, 12), np.float32)
    c4[:, 0:4] = cc2_w[:, 0:4, 0, 0].T
    c4[:, 4:8] = cc2_w[:, 4:8, 0, 0].T
    c4[:, 8] = cc2_b

    shared = {"cls": cls_d, "wcst": wcst, "rcst": rcst, "ccst": ccst, "c4": c4}

    in_maps = []
    for c in range(NCORES):
        scls_pad = np.zeros((IN_DIM, MSH + 2), np.float32)
        lo, hi = c * MSH, (c + 1) * MSH
        scls_pad[:, 1:MSH + 1] = scls_full[:, lo:hi]
        if lo > 0:
            scls_pad[:, 0] = scls_full[:, lo - 1]
        if hi < S2:
            scls_pad[:, MSH + 1] = scls_full[:, hi]

        inp_pad = np.zeros((IN_DIM, QSH + 2), np.float32)
        qlo, qhi = c * QSH, (c + 1) * QSH
        inp_pad[:, 1:QSH + 1] = inp_full[:, qlo:qhi]
        if qlo > 0:
            inp_pad[:, 0] = inp_full[:, qlo - 1]
        if qhi < HW:
            inp_pad[:, QSH + 1] = inp_full[:, qhi]

        in_maps.append(dict(shared, scls=scls_pad,
                            inp=_chunk_with_halo(inp_pad, 4)))
    return in_maps


_PROGRAM = None


def kernel(**inputs) -> tuple[np.ndarray, np.ndarray]:
    global _PROGRAM
    if _PROGRAM is None:
        _PROGRAM = build_program()
    in_maps = _prep_in_maps(inputs)
    r = run_bass_kernel_spmd(_PROGRAM, in_maps, list(range(NCORES)))
    memory_map = np.concatenate([r.results[c]["mm_out"] for c in range(NCORES)], 0)
    res = np.concatenate([r.results[c]["res_out"] for c in range(NCORES)], 1)
    result_map = res.reshape(1, NCLS, H, W)
    return result_map, memory_map


# revision 57
# speedup vs baseline: 1.6992x; 1.6992x over previous
"""Trainium2 Bass kernel for nn_Custom_Self_Attention_28621662060731.

Reference semantics (all "3x3 convs" act on width-1 reshaped tensors, so they
are 3-tap 1D convs along the flattened position axis):

    k_i  = conv3(class_input)        [64, 4096]
    sk_i = conv3(saved_class_input)  [64, 4096]
    q_i  = conv3(input)              [64, 16384]
    memory_map = sigmoid(c1 * cos(sk1,k1) + c2 * cos(sk2,k2) + cb1)  [4096, 4096]
    r_i  = onehot @ cos(k_i, q_i) / counts                            [4, 16384]
    result_map = sigmoid(cc2 @ [r1;r2] + cb2)                         [1,4,128,128]

Key algebraic collapse: the [4096, 16384] attention maps are never
materialized.  r_i = (onehot/counts @ k_i_norm^T) @ q_i_norm, a [4,64]
matrix applied to the normalized queries.

Sharding: memory_map rows (S2) and query pixels (HW) both sharded 8-way;
keys/weights replicated.  Each core computes a [512, 4096] memory_map shard
and a [4, 2048] result shard.  No collectives.

Scheduling notes (engine order == emission order):
 - key chunks are software-pipelined: chunk n's conv matmuls are emitted
   before chunk n-1's normalization, hiding the PE->ACT->PE->DVE->ACT->DVE
   chain latency.
 - the memory-map matmul block for key chunk n is emitted at lag 2, so the
   PE never waits on the normalization chain.
 - the A-matrix path transposes normalized keys via a single bf16
   DMA-transpose per conv (fp32 PE transposes are ~275ns each x64).
 - PSUM accumulation groups are never interleaved with other PE work
   (hardware crashes otherwise).
"""

import numpy as np

import concourse.bass as bass
import concourse.bacc as bacc
import concourse.mybir as mybir
from concourse.tile import TileContext, add_dep_helper
from concourse.bass_utils import run_bass_kernel_spmd

NCORES = 8
IN_DIM, ATTN, NCLS = 128, 64, 4
H = W = 128
HW = H * W                 # 16384
S1 = S2 = 4096
QSH = HW // NCORES         # 2048 query pixels per core
MSH = S2 // NCORES         # 512 memory-map rows per core
CH = 512                   # column chunk (one PSUM bank of fp32)
F32 = mybir.dt.float32
F32R = mybir.dt.float32r
BF16 = mybir.dt.bfloat16
AF = mybir.ActivationFunctionType
MULT = mybir.AluOpType.mult


def _r(ap):
    """Reinterpret an fp32 AP as float32r: same bits, 4x matmul throughput
    at N>=256 (TF32-like multiply precision)."""
    return ap.bitcast(F32R)


def build_program(stage: int = 99) -> bass.Bass:
    """stage limits how much of the pipeline is emitted (for HW bisection):
    1: keys only   2: +A/B   3: +memory map   99: full kernel."""
    nc = bacc.Bacc()

    # halo-padded inputs: cls in two overlapping halves of 4 chunks each
    cls = nc.declare_dram_parameter("cls", [128, 2 * (4 * CH + 2)], F32R,
                                    isOutput=False)
    scls = nc.declare_dram_parameter("scls", [128, CH + 2], F32R, isOutput=False)
    inp = nc.declare_dram_parameter("inp", [128, 4 * CH + 2], F32R, isOutput=False)
    # packed constants, see _prep_in_maps for layout
    wcst = nc.declare_dram_parameter("wcst", [128, 640], F32R, isOutput=False)
    mctb = nc.declare_dram_parameter("mctb", [128, 128], BF16, isOutput=False)
    rcst = nc.declare_dram_parameter("rcst", [1, 640], F32R, isOutput=False)
    ccst = nc.declare_dram_parameter("ccst", [128, 4], F32, isOutput=False)
    c4 = nc.declare_dram_parameter("c4", [4, 12], F32, isOutput=False)

    mm_out = nc.declare_dram_parameter("mm_out", [MSH, S1], F32, isOutput=True)
    res_out = nc.declare_dram_parameter("res_out", [4, QSH], F32, isOutput=True)

    with TileContext(nc) as tc:
        with (
            tc.tile_pool(name="const", bufs=1) as constp,
            tc.tile_pool(name="io", bufs=1) as iop,
            tc.tile_pool(name="work", bufs=3) as wp,
            tc.tile_pool(name="mmo", bufs=1) as mmop,
            tc.tile_pool(name="pacc", bufs=1, space="PSUM") as paccp,
            tc.tile_pool(name="praw", bufs=2, space="PSUM") as prawp,
            tc.tile_pool(name="pnorm", bufs=1, space="PSUM") as pnp,
            tc.tile_pool(name="pbig", bufs=2, space="PSUM") as pbp,
        ):
            # DMA queue assignment spreads dispatch cost: inputs on the
            # gpsimd (SWDGE) queue, small consts on the scalar queue, stores on
            # sync + scalar (both HWDGE)
            wcst_s = constp.tile([128, 640], F32R, name="wcst_s")
            nc.sync.dma_start(out=wcst_s, in_=wcst[:, :])
            rcst_s = constp.tile([1, 640], F32R, name="rcst_s")
            nc.sync.dma_start(out=rcst_s, in_=rcst[:, :])
            ccst_s = constp.tile([128, 4], F32, name="ccst_s")
            nc.sync.dma_start(out=ccst_s, in_=ccst[:, :])

            HCH = 4 * CH + 2
            scls_s = iop.tile([128, CH + 2], F32R, name="scls_s")
            nc.gpsimd.dma_start(out=scls_s, in_=scls[:, :])
            cls_h = []
            for hh in range(2):
                t = iop.tile([128, HCH], F32R, name=f"cls_h{hh}")
                nc.gpsimd.dma_start(out=t, in_=cls[:, HCH * hh:HCH * (hh + 1)])
                cls_h.append(t)
            inp_s = iop.tile([128, HCH], F32R, name="inp_s")
            nc.gpsimd.dma_start(out=inp_s, in_=inp[:, :])
            mctb_s = constp.tile([128, 128], BF16, name="mctb_s")
            nc.gpsimd.dma_start(out=mctb_s, in_=mctb[:, :])
            c4_s = constp.tile([4, 12], F32, name="c4_s")
            nc.gpsimd.dma_start(out=c4_s, in_=c4[:, :])

            # chunk views: (tile, column offset of the chunk's halo start)
            cls_t = [(cls_h[n // 4], CH * (n % 4)) for n in range(8)]
            inp_t = [(inp_s, CH * m) for m in range(4)]

            # constant views
            def w1t(t):
                return wcst_s[:, 64 * t:64 * t + 64]

            def w2t(t):
                return wcst_s[:, 192 + 64 * t:192 + 64 * t + 64]

            bd = wcst_s[:, 384:512]         # [128, 128] blockdiag ones
            bstack = rcst_s[0:1, 0:128]     # b1 ++ b2
            ones512 = rcst_s[0:1, 128:640]
            cvec = ccst_s[:, 0:1]           # c1 rows 0:64, c2 rows 64:128
            cb1v = ccst_s[:, 1:2]           # cc1_b broadcast
            # conv biases at the base partition of the conv half they feed
            bv = [ccst_s[0:64, 2:3], ccst_s[64:128, 2:3]]
            cc2aT = c4_s[0:4, 0:4]
            cc2bT = c4_s[0:4, 4:8]
            cb2v = c4_s[0:4, 8:9]

            # stacked normalized keys/queries: rows 0:64 conv1, 64:128 conv2
            kc = iop.tile([128, S1], F32R, name="kc")
            skc = iop.tile([128, CH], F32R, name="skc")
            qc = iop.tile([128, QSH], F32R, name="qc")
            last_sqrt = [None]

            def key_convs(src_off):
                """conv both attn heads on 512 positions.  Each head gets its
                own base-0 PSUM tile: fp32r matmuls reject sub-partition
                accumulation groups.  Conv bias is folded into the Square /
                normalize ops downstream."""
                src, off = src_off
                pr1 = prawp.tile([64, CH], F32, name="pr1", tag="pr1")
                pr2 = prawp.tile([64, CH], F32, name="pr2", tag="pr2")
                for t in range(3):
                    nc.tensor.matmul(pr1, lhsT=w1t(t),
                                     rhs=src[:, off + t:off + t + CH],
                                     start=(t == 0), stop=(t == 2))
                for t in range(3):
                    nc.tensor.matmul(pr2, lhsT=w2t(t),
                                     rhs=src[:, off + t:off + t + CH],
                                     start=(t == 0), stop=(t == 2))
                return (pr1, pr2)

            def norm_front(pr):
                """squares + sumsq + reciprocal for one conv chunk."""
                pr1, pr2 = pr
                sq = wp.tile([128, CH], F32R, name="sq", tag="sq")
                nc.scalar.activation(sq[0:64, :], pr1, AF.Square, bias=bv[0])
                nc.scalar.activation(sq[64:128, :], pr2, AF.Square, bias=bv[1])
                # per-column sum of squares broadcast to all partitions
                pn = pnp.tile([128, CH], F32, name="pn", tag="pn")
                nc.tensor.matmul(pn, lhsT=bd, rhs=sq,
                                 start=True, stop=True)
                rec = wp.tile([128, CH], F32, name="rec", tag="rec", bufs=4)
                nc.vector.reciprocal(rec, pn)
                return rec

            def norm_back(pr, rec, dst, col0, scale_cvec):
                """1/sqrt + bias-add + scale into dst columns."""
                pr1, pr2 = pr
                inv = wp.tile([128, CH], F32, name="inv", tag="inv")
                sqrt_i = nc.scalar.sqrt(inv, rec)  # 1/sqrt(ss) = sqrt(1/ss)
                last_sqrt[0] = sqrt_i
                if scale_cvec:
                    # fold the cc1 coefficients into the inverse norms
                    inv2 = wp.tile([128, CH], F32, name="inv2", tag="inv2")
                    nc.vector.tensor_scalar_mul(inv2, inv, cvec)
                    inv = inv2
                for jj, prj in ((0, pr1), (1, pr2)):
                    nc.vector.scalar_tensor_tensor(
                        out=dst[64 * jj:64 * jj + 64, col0:col0 + CH],
                        in0=prj, scalar=bv[jj],
                        in1=inv[64 * jj:64 * jj + 64, :],
                        op0=mybir.AluOpType.add, op1=MULT)

            # 4 persistent row-block staging tiles; stored in halves
            mm_sb = [mmop.tile([128, S1], F32, name=f"mm_sb{m}", tag=f"mm{m}")
                     for m in range(4)]

            def mm_block(n):
                """memory-map block for key columns [512n, 512n+512)."""
                for m in range(4):
                    pb = pbp.tile([128, CH], F32, name="pb", tag="pb")
                    nc.tensor.matmul(pb,
                                     lhsT=skc[:, 128 * m:128 * m + 128],
                                     rhs=kc[:, CH * n:CH * n + CH],
                                     start=True, stop=True)
                    sig = nc.scalar.activation(mm_sb[m][:, CH * n:CH * n + CH],
                                               pb, AF.Sigmoid, bias=cb1v)
                    # keep every Sigmoid after the last Sqrt: the scheduler
                    # otherwise interleaves them and thrashes the ACT
                    # function table (667ns reload per switch)
                    if last_sqrt[0] is not None:
                        add_dep_helper(sig.ins, last_sqrt[0].ins,
                                       reason="ACT table: sigmoid after sqrt")
                    if n == 3 or n == 7:
                        hh = n // 4
                        eng = nc.gpsimd if m < 2 else nc.sync
                        eng.dma_start(
                            out=mm_out[128 * m:128 * m + 128,
                                       2048 * hh:2048 * (hh + 1)],
                            in_=mm_sb[m][:, 2048 * hh:2048 * (hh + 1)])

            # ---- conv-chunk pipeline: lag-2 emission (PE convs(n) |
            # front(n-1) | back(n-2)) keeps every engine fed.  ACT only sees
            # Square/Sqrt inside a pipeline run (one table set).
            def chunk_pipeline(chunks):
                n_c = len(chunks)
                prs = [None] * n_c
                recs = [None] * n_c

                def back(i):
                    _, dst, col0, sc = chunks[i]
                    norm_back(prs[i], recs[i], dst, col0, sc)
                    prs[i] = None
                    recs[i] = None

                for i in range(n_c):
                    prs[i] = key_convs(chunks[i][0])
                    if i >= 1:
                        recs[i - 1] = norm_front(prs[i - 1])
                    if i >= 2:
                        back(i - 2)
                recs[n_c - 1] = norm_front(prs[n_c - 1])
                for i in range(max(0, n_c - 2), n_c):
                    back(i)

            # keys + saved keys
            chunk_pipeline([((scls_s, 0), skc, 0, True)] +
                           [(cls_t[n], kc, CH * n, False) for n in range(8)])

            if stage >= 2:
                # ---- A matrices: A_j = (onehot/counts) @ k_j_norm^T ----
                # transpose normalized keys via one bf16 DMA-transpose per conv
                kcb = iop.tile([128, S1], BF16, name="kcb")
                nc.vector.tensor_copy(kcb, kc)
                knTb = []
                for j in range(2):
                    t3 = iop.tile([128, 32, 64], BF16, name=f"knTb{j}")
                    nc.sync.dma_start_transpose(t3, kcb[64 * j:64 * j + 64, :])
                    knTb.append(t3.rearrange("p i a -> p (i a)"))
                # two sequential uninterrupted accumulation groups
                A_ps = paccp.tile([4, 128], F32, name="A_ps")
                for j in range(2):
                    for i in range(32):
                        nc.tensor.matmul(A_ps[0:4, 64 * j:64 * j + 64],
                                         lhsT=mctb_s[:, 4 * i:4 * i + 4],
                                         rhs=knTb[j][:, 64 * i:64 * i + 64],
                                         start=(i == 0), stop=(i == 31),
                                         skip_group_check=True)

                # B_j^T = A_j^T @ cc2_j^T, stacked [128, 4]
                A_sb = iop.tile([4, 128], F32, name="A_sb")
                nc.scalar.copy(A_sb, A_ps)
                B_ps = pbp.tile([128, 4], F32, name="B_ps", tag="pb")
                nc.tensor.matmul(B_ps[0:64, :], lhsT=A_sb[0:4, 0:64],
                                 rhs=cc2aT, start=True, stop=True,
                                 skip_group_check=True)
                nc.tensor.matmul(B_ps[64:128, :], lhsT=A_sb[0:4, 64:128],
                                 rhs=cc2bT, start=True, stop=True,
                                 skip_group_check=True)
                Bc_sb = iop.tile([128, 4], F32R, name="Bc_sb")
                nc.vector.tensor_copy(Bc_sb, B_ps)
            if stage == 2:
                nc.sync.dma_start(out=res_out[0:4, 0:128], in_=A_sb)

            # ---- memory map: sigmoid(skc^T @ kc + cb1)  [512, 4096] ----
            # sigmoids are gated on the key phase's last sqrt (ACT table)
            if stage >= 3:
                for n in range(8):
                    mm_block(n)

            # ---- queries for this core's 2048 pixels ----
            if stage >= 4:
                chunk_pipeline([(inp_t[m], qc, CH * m, False)
                                for m in range(4)])

                # ---- result shard: sigmoid(Bc^T @ qc + cb2)  [4, 2048] ----
                res_sb = iop.tile([4, QSH], F32, name="res_sb")
                for m in range(4):
                    pres = pbp.tile([4, CH], F32, name="pres", tag="pb")
                    nc.tensor.matmul(pres, lhsT=Bc_sb[:, 0:4],
                                     rhs=qc[:, CH * m:CH * m + CH],
                                     start=True, stop=True)
                    sig = nc.scalar.activation(res_sb[:, CH * m:CH * m + CH],
                                               pres, AF.Sigmoid, bias=cb2v)
                    add_dep_helper(sig.ins, last_sqrt[0].ins,
                                   reason="ACT table: sigmoid after sqrt")
                nc.scalar.dma_start(out=res_out[:, :], in_=res_sb)

            if stage == 1:
                nc.sync.dma_start(out=mm_out[0:128, :], in_=kc)
                nc.sync.dma_start(out=mm_out[128:256, 0:CH], in_=skc)

    nc.finalize()
    return nc


def _chunk_with_halo(x_pad: np.ndarray, nchunks: int) -> np.ndarray:
    """x_pad [128, L+2] zero-padded; -> [128, nchunks*(CH+2)] where chunk n
    is x_pad[:, CH*n : CH*n+CH+2]."""
    out = np.empty((128, nchunks * (CH + 2)), np.float32)
    for n in range(nchunks):
        out[:, (CH + 2) * n:(CH + 2) * (n + 1)] = x_pad[:, CH * n:CH * n + CH + 2]
    return out


def _prep_in_maps(inputs: dict) -> list[dict]:
    import ml_dtypes

    inp_full = np.ascontiguousarray(
        inputs["input"].reshape(IN_DIM, HW), dtype=np.float32)
    cls_full = np.ascontiguousarray(
        inputs["class_input"].reshape(IN_DIM, S1), dtype=np.float32)
    scls_full = np.ascontiguousarray(
        inputs["saved_class_input"].reshape(IN_DIM, S2), dtype=np.float32)
    w1 = np.asarray(inputs["w1"], np.float32)
    w2 = np.asarray(inputs["w2"], np.float32)
    b1 = np.asarray(inputs["b1"], np.float32)
    b2 = np.asarray(inputs["b2"], np.float32)
    cc1_w = np.asarray(inputs["cc1_w"], np.float32)
    cc1_b = np.asarray(inputs["cc1_b"], np.float32)
    cc2_w = np.asarray(inputs["cc2_w"], np.float32)
    cc2_b = np.asarray(inputs["cc2_b"], np.float32)
    lbl = np.asarray(inputs["class_label"])

    cls_pad = np.zeros((IN_DIM, S1 + 2), np.float32)
    cls_pad[:, 1:S1 + 1] = cls_full
    # two overlapping halves of 4 chunks each: half h = padded cols [2048h, +2050)
    cls_d = np.concatenate([cls_pad[:, 0:2050], cls_pad[:, 2048:4098]], axis=1)

    onehot = (lbl[None, :] == np.arange(NCLS)[:, None]).astype(np.float32)
    msc = onehot / onehot.sum(1, keepdims=True)          # [4, 4096]

    wcst = np.zeros((128, 640), np.float32)
    for t in range(3):
        wcst[:, 64 * t:64 * t + 64] = w1[:, :, t, 1].T
        wcst[:, 192 + 64 * t:192 + 64 * t + 64] = w2[:, :, t, 1].T
    wcst[0:64, 384:448] = 1.0     # blockdiag ones
    wcst[64:128, 448:512] = 1.0

    # mctb[p, 4i+c] = msc[c, 128i+p]  (bf16, feeds the A-matrix matmuls)
    mct = msc.reshape(NCLS, 32, 128).transpose(2, 1, 0).reshape(128, 128)
    mctb = mct.astype(ml_dtypes.bfloat16)

    rcst = np.zeros((1, 640), np.float32)
    rcst[0, 0:64] = b1
    rcst[0, 64:128] = b2
    rcst[0, 128:640] = 1.0

    ccst = np.zeros((128, 4), np.float32)
    ccst[0:64, 0] = cc1_w[0, 0, 0, 0]
    ccst[64:128, 0] = cc1_w[0, 1, 0, 0]
    ccst[:, 1] = cc1_b[0]
    ccst[0:64, 2] = b1
    ccst[64:128, 2] = b2

    c4 = np.zeros((4, 12), np.float32)
    c4[:, 0:4] = cc2_w[:, 0:4, 0, 0].T
    c4[:, 4:8] = cc2_w[:, 4:8, 0, 0].T
    c4[:, 8] = cc2_b

    shared = {"cls": cls_d, "wcst": wcst, "mctb": mctb, "rcst": rcst,
              "ccst": ccst, "c4": c4}

    in_maps = []
    for c in range(NCORES):
        scls_pad = np.zeros((IN_DIM, MSH + 2), np.float32)
        lo, hi = c * MSH, (c + 1) * MSH
        scls_pad[:, 1:MSH + 1] = scls_full[:, lo:hi]
        if lo > 0:
            scls_pad[:, 0] = scls_full[:, lo - 1]
        if hi < S2:
            scls_pad[:, MSH + 1] = scls_full[:, hi]

        inp_pad = np.zeros((IN_DIM, QSH + 2), np.float32)
        qlo, qhi = c * QSH, (c + 1) * QSH
        inp_pad[:, 1:QSH + 1] = inp_full[:, qlo:qhi]
        if qlo > 0:
            inp_pad[:, 0] = inp_full[:, qlo - 1]
        if qhi < HW:
            inp_pad[:, QSH + 1] = inp_full[:, qhi]

        in_maps.append(dict(shared, scls=scls_pad, inp=inp_pad))
    return in_maps


_PROGRAM = None


def kernel(**inputs) -> tuple[np.ndarray, np.ndarray]:
    global _PROGRAM
    if _PROGRAM is None:
        _PROGRAM = build_program()
    in_maps = _prep_in_maps(inputs)
    r = run_bass_kernel_spmd(_PROGRAM, in_maps, list(range(NCORES)))
    memory_map = np.concatenate([r.results[c]["mm_out"] for c in range(NCORES)], 0)
    res = np.concatenate([r.results[c]["res_out"] for c in range(NCORES)], 1)
    result_map = res.reshape(1, NCLS, H, W)
    return result_map, memory_map


# revision 59
# speedup vs baseline: 68098.2380x; 40075.8576x over previous
"""Trainium2 Bass kernel for nn_Custom_Self_Attention_28621662060731.

Reference semantics (all "3x3 convs" act on width-1 reshaped tensors, so they
are 3-tap 1D convs along the flattened position axis):

    k_i  = conv3(class_input)        [64, 4096]
    sk_i = conv3(saved_class_input)  [64, 4096]
    q_i  = conv3(input)              [64, 16384]
    memory_map = sigmoid(c1 * cos(sk1,k1) + c2 * cos(sk2,k2) + cb1)  [4096, 4096]
    r_i  = onehot @ cos(k_i, q_i) / counts                            [4, 16384]
    result_map = sigmoid(cc2 @ [r1;r2] + cb2)                         [1,4,128,128]

Key algebraic collapse: the [4096, 16384] attention maps are never
materialized.  r_i = (onehot/counts @ k_i_norm^T) @ q_i_norm, a [4,64]
matrix applied to the normalized queries.

Sharding: memory_map rows (S2) and query pixels (HW) both sharded 8-way;
keys/weights replicated.  Each core computes a [512, 4096] memory_map shard
and a [4, 2048] result shard.  No collectives.

Scheduling notes (engine order == emission order):
 - key chunks are software-pipelined: chunk n's conv matmuls are emitted
   before chunk n-1's normalization, hiding the PE->ACT->PE->DVE->ACT->DVE
   chain latency.
 - the memory-map matmul block for key chunk n is emitted at lag 2, so the
   PE never waits on the normalization chain.
 - the A-matrix path transposes normalized keys via a single bf16
   DMA-transpose per conv (fp32 PE transposes are ~275ns each x64).
 - PSUM accumulation groups are never interleaved with other PE work
   (hardware crashes otherwise).
"""

import numpy as np

import concourse.bass as bass
import concourse.bacc as bacc
import concourse.mybir as mybir
from concourse.tile import TileContext, add_dep_helper
from concourse.bass_utils import run_bass_kernel_spmd

NCORES = 8
IN_DIM, ATTN, NCLS = 128, 64, 4
H = W = 128
HW = H * W                 # 16384
S1 = S2 = 4096
QSH = HW // NCORES         # 2048 query pixels per core
MSH = S2 // NCORES         # 512 memory-map rows per core
CH = 512                   # column chunk (one PSUM bank of fp32)
F32 = mybir.dt.float32
F32R = mybir.dt.float32r
BF16 = mybir.dt.bfloat16
AF = mybir.ActivationFunctionType
MULT = mybir.AluOpType.mult


def _r(ap):
    """Reinterpret an fp32 AP as float32r: same bits, 4x matmul throughput
    at N>=256 (TF32-like multiply precision)."""
    return ap.bitcast(F32R)


def build_program(stage: int = 99) -> bass.Bass:
    """stage limits how much of the pipeline is emitted (for HW bisection):
    1: keys only   2: +A/B   3: +memory map   99: full kernel."""
    nc = bacc.Bacc()

    # halo-padded inputs: cls in two overlapping halves of 4 chunks each
    cls = nc.declare_dram_parameter("cls", [128, 2 * (4 * CH + 2)], F32R,
                                    isOutput=False)
    scls = nc.declare_dram_parameter("scls", [128, CH + 2], F32R, isOutput=False)
    inp = nc.declare_dram_parameter("inp", [128, 4 * CH + 2], F32R, isOutput=False)
    # packed constants, see _prep_in_maps for layout
    wcst = nc.declare_dram_parameter("wcst", [128, 640], F32R, isOutput=False)
    mctb = nc.declare_dram_parameter("mctb", [128, 128], BF16, isOutput=False)
    rcst = nc.declare_dram_parameter("rcst", [1, 640], F32R, isOutput=False)
    ccst = nc.declare_dram_parameter("ccst", [128, 4], F32, isOutput=False)
    c4 = nc.declare_dram_parameter("c4", [4, 12], F32, isOutput=False)

    mm_out = nc.declare_dram_parameter("mm_out", [MSH, S1], F32, isOutput=True)
    res_out = nc.declare_dram_parameter("res_out", [4, QSH], F32, isOutput=True)

    with TileContext(nc) as tc:
        with (
            tc.tile_pool(name="const", bufs=1) as constp,
            tc.tile_pool(name="io", bufs=1) as iop,
            tc.tile_pool(name="work", bufs=3) as wp,
            tc.tile_pool(name="mmo", bufs=1) as mmop,
            tc.tile_pool(name="pacc", bufs=1, space="PSUM") as paccp,
            tc.tile_pool(name="praw", bufs=2, space="PSUM") as prawp,
            tc.tile_pool(name="pnorm", bufs=1, space="PSUM") as pnp,
            tc.tile_pool(name="pbig", bufs=2, space="PSUM") as pbp,
        ):
            # DMA queue assignment spreads dispatch cost: inputs on the
            # gpsimd (SWDGE) queue, small consts on the scalar queue, stores on
            # sync + scalar (both HWDGE)
            wcst_s = constp.tile([128, 640], F32R, name="wcst_s")
            nc.sync.dma_start(out=wcst_s, in_=wcst[:, :])
            rcst_s = constp.tile([1, 640], F32R, name="rcst_s")
            nc.sync.dma_start(out=rcst_s, in_=rcst[:, :])
            ccst_s = constp.tile([128, 4], F32, name="ccst_s")
            nc.sync.dma_start(out=ccst_s, in_=ccst[:, :])

            HCH = 4 * CH + 2
            scls_s = iop.tile([128, CH + 2], F32R, name="scls_s")
            nc.gpsimd.dma_start(out=scls_s, in_=scls[:, :])
            cls_h = []
            for hh in range(2):
                t = iop.tile([128, HCH], F32R, name=f"cls_h{hh}")
                nc.gpsimd.dma_start(out=t, in_=cls[:, HCH * hh:HCH * (hh + 1)])
                cls_h.append(t)
            inp_s = iop.tile([128, HCH], F32R, name="inp_s")
            nc.gpsimd.dma_start(out=inp_s, in_=inp[:, :])
            mctb_s = constp.tile([128, 128], BF16, name="mctb_s")
            nc.gpsimd.dma_start(out=mctb_s, in_=mctb[:, :])
            c4_s = constp.tile([4, 12], F32, name="c4_s")
            nc.gpsimd.dma_start(out=c4_s, in_=c4[:, :])

            # chunk views: (tile, column offset of the chunk's halo start)
            cls_t = [(cls_h[n // 4], CH * (n % 4)) for n in range(8)]
            inp_t = [(inp_s, CH * m) for m in range(4)]

            # constant views
            def w1t(t):
                return wcst_s[:, 64 * t:64 * t + 64]

            def w2t(t):
                return wcst_s[:, 192 + 64 * t:192 + 64 * t + 64]

            bd = wcst_s[:, 384:512]         # [128, 128] blockdiag ones
            bstack = rcst_s[0:1, 0:128]     # b1 ++ b2
            ones512 = rcst_s[0:1, 128:640]
            cvec = ccst_s[:, 0:1]           # c1 rows 0:64, c2 rows 64:128
            cb1v = ccst_s[:, 1:2]           # cc1_b broadcast
            # conv biases at the base partition of the conv half they feed
            bv = [ccst_s[0:64, 2:3], ccst_s[64:128, 2:3]]
            cc2aT = c4_s[0:4, 0:4]
            cc2bT = c4_s[0:4, 4:8]
            cb2v = c4_s[0:4, 8:9]

            # stacked normalized keys/queries: rows 0:64 conv1, 64:128 conv2
            kc = iop.tile([128, S1], F32R, name="kc")
            skc = iop.tile([128, CH], F32R, name="skc")
            qc = iop.tile([128, QSH], F32R, name="qc")
            last_sqrt = [None]

            def key_convs(src_off):
                """conv both attn heads on 512 positions.  Each head gets its
                own base-0 PSUM tile: fp32r matmuls reject sub-partition
                accumulation groups.  Conv bias is folded into the Square /
                normalize ops downstream."""
                src, off = src_off
                pr1 = prawp.tile([64, CH], F32, name="pr1", tag="pr1")
                pr2 = prawp.tile([64, CH], F32, name="pr2", tag="pr2")
                for t in range(3):
                    nc.tensor.matmul(pr1, lhsT=w1t(t),
                                     rhs=src[:, off + t:off + t + CH],
                                     start=(t == 0), stop=(t == 2))
                for t in range(3):
                    nc.tensor.matmul(pr2, lhsT=w2t(t),
                                     rhs=src[:, off + t:off + t + CH],
                                     start=(t == 0), stop=(t == 2))
                return (pr1, pr2)

            def norm_front(pr):
                """squares + sumsq + reciprocal for one conv chunk."""
                pr1, pr2 = pr
                sq = wp.tile([128, CH], F32R, name="sq", tag="sq")
                nc.scalar.activation(sq[0:64, :], pr1, AF.Square, bias=bv[0])
                nc.scalar.activation(sq[64:128, :], pr2, AF.Square, bias=bv[1])
                # per-column sum of squares broadcast to all partitions
                pn = pnp.tile([128, CH], F32, name="pn", tag="pn")
                nc.tensor.matmul(pn, lhsT=bd, rhs=sq,
                                 start=True, stop=True)
                rec = wp.tile([128, CH], F32, name="rec", tag="rec", bufs=4)
                nc.vector.reciprocal(rec, pn)
                return rec

            def norm_back(pr, rec, dst, col0, scale_cvec):
                """1/sqrt + bias-add + scale into dst columns."""
                pr1, pr2 = pr
                inv = wp.tile([128, CH], F32, name="inv", tag="inv")
                sqrt_i = nc.scalar.sqrt(inv, rec)  # 1/sqrt(ss) = sqrt(1/ss)
                last_sqrt[0] = sqrt_i
                if scale_cvec:
                    # fold the cc1 coefficients into the inverse norms
                    inv2 = wp.tile([128, CH], F32, name="inv2", tag="inv2")
                    nc.vector.tensor_scalar_mul(inv2, inv, cvec)
                    inv = inv2
                for jj, prj in ((0, pr1), (1, pr2)):
                    nc.vector.scalar_tensor_tensor(
                        out=dst[64 * jj:64 * jj + 64, col0:col0 + CH],
                        in0=prj, scalar=bv[jj],
                        in1=inv[64 * jj:64 * jj + 64, :],
                        op0=mybir.AluOpType.add, op1=MULT)

            # 4 persistent row-block staging tiles; stored in halves
            mm_sb = [mmop.tile([128, S1], F32, name=f"mm_sb{m}", tag=f"mm{m}")
                     for m in range(4)]

            def mm_block(n):
                """memory-map block for key columns [512n, 512n+512)."""
                for m in range(4):
                    pb = pbp.tile([128, CH], F32, name="pb", tag="pb")
                    nc.tensor.matmul(pb,
                                     lhsT=skc[:, 128 * m:128 * m + 128],
                                     rhs=kc[:, CH * n:CH * n + CH],
                                     start=True, stop=True)
                    sig = nc.scalar.activation(mm_sb[m][:, CH * n:CH * n + CH],
                                               pb, AF.Sigmoid, bias=cb1v)
                    # keep every Sigmoid after the last Sqrt: the scheduler
                    # otherwise interleaves them and thrashes the ACT
                    # function table (667ns reload per switch)
                    if last_sqrt[0] is not None:
                        add_dep_helper(sig.ins, last_sqrt[0].ins,
                                       reason="ACT table: sigmoid after sqrt")
                    if n == 3 or n == 7:
                        hh = n // 4
                        eng = nc.gpsimd if m < 2 else nc.sync
                        eng.dma_start(
                            out=mm_out[128 * m:128 * m + 128,
                                       2048 * hh:2048 * (hh + 1)],
                            in_=mm_sb[m][:, 2048 * hh:2048 * (hh + 1)])

            # ---- conv-chunk pipeline: lag-2 emission (PE convs(n) |
            # front(n-1) | back(n-2)) keeps every engine fed.  ACT only sees
            # Square/Sqrt inside a pipeline run (one table set).
            def chunk_pipeline(chunks):
                n_c = len(chunks)
                prs = [None] * n_c
                recs = [None] * n_c

                def back(i):
                    _, dst, col0, sc = chunks[i]
                    norm_back(prs[i], recs[i], dst, col0, sc)
                    prs[i] = None
                    recs[i] = None

                for i in range(n_c):
                    prs[i] = key_convs(chunks[i][0])
                    if i >= 1:
                        recs[i - 1] = norm_front(prs[i - 1])
                    if i >= 2:
                        back(i - 2)
                recs[n_c - 1] = norm_front(prs[n_c - 1])
                for i in range(max(0, n_c - 2), n_c):
                    back(i)

            # keys + saved keys
            chunk_pipeline([((scls_s, 0), skc, 0, True)] +
                           [(cls_t[n], kc, CH * n, False) for n in range(8)])

            if stage >= 2:
                # ---- A matrices: A_j = (onehot/counts) @ k_j_norm^T ----
                # transpose normalized keys via one bf16 DMA-transpose per conv
                kcb = iop.tile([128, S1], BF16, name="kcb")
                nc.vector.tensor_copy(kcb, kc)
                knTb = []
                for j in range(2):
                    t3 = iop.tile([128, 32, 64], BF16, name=f"knTb{j}")
                    nc.sync.dma_start_transpose(t3, kcb[64 * j:64 * j + 64, :])
                    knTb.append(t3.rearrange("p i a -> p (i a)"))
                # two sequential uninterrupted accumulation groups
                A_ps = paccp.tile([4, 128], F32, name="A_ps")
                for j in range(2):
                    for i in range(32):
                        nc.tensor.matmul(A_ps[0:4, 64 * j:64 * j + 64],
                                         lhsT=mctb_s[:, 4 * i:4 * i + 4],
                                         rhs=knTb[j][:, 64 * i:64 * i + 64],
                                         start=(i == 0), stop=(i == 31),
                                         skip_group_check=True)

                # B_j^T = A_j^T @ cc2_j^T, stacked [128, 4]
                A_sb = iop.tile([4, 128], F32, name="A_sb")
                nc.scalar.copy(A_sb, A_ps)
                B_ps = pbp.tile([128, 4], F32, name="B_ps", tag="pb")
                nc.tensor.matmul(B_ps[0:64, :], lhsT=A_sb[0:4, 0:64],
                                 rhs=cc2aT, start=True, stop=True,
                                 skip_group_check=True)
                nc.tensor.matmul(B_ps[64:128, :], lhsT=A_sb[0:4, 64:128],
                                 rhs=cc2bT, start=True, stop=True,
                                 skip_group_check=True)
                Bc_sb = iop.tile([128, 4], F32R, name="Bc_sb")
                nc.vector.tensor_copy(Bc_sb, B_ps)
            if stage == 2:
                nc.sync.dma_start(out=res_out[0:4, 0:128], in_=A_sb)

            # ---- memory map: sigmoid(skc^T @ kc + cb1)  [512, 4096] ----
            # sigmoids are gated on the key phase's last sqrt (ACT table)
            if stage >= 3:
                for n in range(8):
                    mm_block(n)

            # ---- queries for this core's 2048 pixels ----
            if stage >= 4:
                chunk_pipeline([(inp_t[m], qc, CH * m, False)
                                for m in range(4)])

                # ---- result shard: sigmoid(Bc^T @ qc + cb2)  [4, 2048] ----
                res_sb = iop.tile([4, QSH], F32, name="res_sb")
                for m in range(4):
                    pres = pbp.tile([4, CH], F32, name="pres", tag="pb")
                    nc.tensor.matmul(pres, lhsT=Bc_sb[:, 0:4],
                                     rhs=qc[:, CH * m:CH * m + CH],
                                     start=True, stop=True)
                    sig = nc.scalar.activation(res_sb[:, CH * m:CH * m + CH],
                                               pres, AF.Sigmoid, bias=cb2v)
                    add_dep_helper(sig.ins, last_sqrt[0].ins,
                                   reason="ACT table: sigmoid after sqrt")
                nc.scalar.dma_start(out=res_out[:, :], in_=res_sb)

            if stage == 1:
                nc.sync.dma_start(out=mm_out[0:128, :], in_=kc)
                nc.sync.dma_start(out=mm_out[128:256, 0:CH], in_=skc)

    nc.finalize()
    return nc


def _chunk_with_halo(x_pad: np.ndarray, nchunks: int) -> np.ndarray:
    """x_pad [128, L+2] zero-padded; -> [128, nchunks*(CH+2)] where chunk n
    is x_pad[:, CH*n : CH*n+CH+2]."""
    out = np.empty((128, nchunks * (CH + 2)), np.float32)
    for n in range(nchunks):
        out[:, (CH + 2) * n:(CH + 2) * (n + 1)] = x_pad[:, CH * n:CH * n + CH + 2]
    return out


def _prep_in_maps(inputs: dict) -> list[dict]:
    import ml_dtypes

    inp_full = np.ascontiguousarray(
        inputs["input"].reshape(IN_DIM, HW), dtype=np.float32)
    cls_full = np.ascontiguousarray(
        inputs["class_input"].reshape(IN_DIM, S1), dtype=np.float32)
    scls_full = np.ascontiguousarray(
        inputs["saved_class_input"].reshape(IN_DIM, S2), dtype=np.float32)
    w1 = np.asarray(inputs["w1"], np.float32)
    w2 = np.asarray(inputs["w2"], np.float32)
    b1 = np.asarray(inputs["b1"], np.float32)
    b2 = np.asarray(inputs["b2"], np.float32)
    cc1_w = np.asarray(inputs["cc1_w"], np.float32)
    cc1_b = np.asarray(inputs["cc1_b"], np.float32)
    cc2_w = np.asarray(inputs["cc2_w"], np.float32)
    cc2_b = np.asarray(inputs["cc2_b"], np.float32)
    lbl = np.asarray(inputs["class_label"])

    cls_pad = np.zeros((IN_DIM, S1 + 2), np.float32)
    cls_pad[:, 1:S1 + 1] = cls_full
    # two overlapping halves of 4 chunks each: half h = padded cols [2048h, +2050)
    cls_d = np.concatenate([cls_pad[:, 0:2050], cls_pad[:, 2048:4098]], axis=1)

    onehot = (lbl[None, :] == np.arange(NCLS)[:, None]).astype(np.float32)
    msc = onehot / onehot.sum(1, keepdims=True)          # [4, 4096]

    wcst = np.zeros((128, 640), np.float32)
    for t in range(3):
        wcst[:, 64 * t:64 * t + 64] = w1[:, :, t, 1].T
        wcst[:, 192 + 64 * t:192 + 64 * t + 64] = w2[:, :, t, 1].T
    wcst[0:64, 384:448] = 1.0     # blockdiag ones
    wcst[64:128, 448:512] = 1.0

    # mctb[p, 4i+c] = msc[c, 128i+p]  (bf16, feeds the A-matrix matmuls)
    mct = msc.reshape(NCLS, 32, 128).transpose(2, 1, 0).reshape(128, 128)
    mctb = mct.astype(ml_dtypes.bfloat16)

    rcst = np.zeros((1, 640), np.float32)
    rcst[0, 0:64] = b1
    rcst[0, 64:128] = b2
    rcst[0, 128:640] = 1.0

    ccst = np.zeros((128, 4), np.float32)
    ccst[0:64, 0] = cc1_w[0, 0, 0, 0]
    ccst[64:128, 0] = cc1_w[0, 1, 0, 0]
    ccst[:, 1] = cc1_b[0]
    ccst[0:64, 2] = b1
    ccst[64:128, 2] = b2

    c4 = np.zeros((4, 12), np.float32)
    c4[:, 0:4] = cc2_w[:, 0:4, 0, 0].T
    c4[:, 4:8] = cc2_w[:, 4:8, 0, 0].T
    c4[:, 8] = cc2_b

    shared = {"cls": cls_d, "wcst": wcst, "mctb": mctb, "rcst": rcst,
              "ccst": ccst, "c4": c4}

    in_maps = []
    for c in range(NCORES):
        scls_pad = np.zeros((IN_DIM, MSH + 2), np.float32)
        lo, hi = c * MSH, (c + 1) * MSH
        scls_pad[:, 1:MSH + 1] = scls_full[:, lo:hi]
        if lo > 0:
            scls_pad[:, 0] = scls_full[:, lo - 1]
        if hi < S2:
            scls_pad[:, MSH + 1] = scls_full[:, hi]

        inp_pad = np.zeros((IN_DIM, QSH + 2), np.float32)
        qlo, qhi = c * QSH, (c + 1) * QSH
        inp_pad[:, 1:QSH + 1] = inp_full[:, qlo:qhi]
        if qlo > 0:
            inp_pad[:, 0] = inp_full[:, qlo - 1]
        if qhi < HW:
            inp_pad[:, QSH + 1] = inp_full[:, qhi]

        in_maps.append(dict(shared, scls=scls_pad, inp=inp_pad))
    return in_maps


_PROGRAM = None


def kernel(**inputs) -> tuple[np.ndarray, np.ndarray]:
    global _PROGRAM
    if _PROGRAM is None:
        _PROGRAM = build_program()
    in_maps = _prep_in_maps(inputs)
    r = run_bass_kernel_spmd(_PROGRAM, in_maps, list(range(NCORES)))
    memory_map = np.concatenate([r.results[c]["mm_out"] for c in range(NCORES)], 0)
    res = np.concatenate([r.results[c]["res_out"] for c in range(NCORES)], 1)
    result_map = res.reshape(1, NCLS, H, W)
    return result_map, memory_map
